# revision 1
# baseline (speedup 1.0000x reference)
"""Trainium2 Bass kernel for nn_ClusterPrediction (DynamicEdgeConv x3 + edge head).

Math (reference):
  3x DynamicEdgeConv: kNN(k=30) in feature space, per-edge MLP on
  [x_i, x_j - x_i] with LeakyReLU(0.2), max aggregation.
  Edge head on canonicalized (sorted) edge_index columns:
  sigmoid(W2 . LRelu(Wh1 . [x_u, x_v] + bh1) + bh2).

Key device-side tricks:
  * Distance ranking key: S_ij = 2 x_i.x_j - |x_j|^2 (row-constant |x_i|^2
    dropped). Computed as ONE matmul with augmented contraction:
    stationary [2*x_q ; 1], moving [x_all ; -|x|^2] (streamed from DRAM in
    512-column chunks).
  * Top-30 via DVE max/max_index/match_replace (top-8 per call):
    - per 512-chunk: pack the chunk-local index (9 bits) into the low
      mantissa bits of the fp32 key (scalar_tensor_tensor:
      (S & 0xFFFFFE00) | iota), then vector.max -> per-chunk top-8
      candidates whose values carry their own indices.
    - phase 2 on the 32*8 candidates: 4 rounds of max / max_index /
      match_replace -> ranks 1..32 descending; ranks 31/32 replaced by a
      duplicate of rank 1 (no-op under max-aggregation).
    - global index = (candidate_pos >> 3)*512 + (bits & 0x1FF).
  * MLP decomposed: W.[x_i ; x_j - x_i] = (Wa-Wb).x_i + Wb.x_j; LeakyReLU
    is monotonic and the +v_i term commutes with max, so
    out_i = LRelu(v_i + b + max_k u_{j_k}) with u = Wb.x. Per-neighbor work
    is just a gather of u^T columns (gpsimd.ap_gather, on-chip) + max.
  * Features sharded over 8 cores (2048 queries each); AllGather of x
    (transposed) between layers.
"""

import numpy as np

import concourse.bacc as bacc
import concourse.bass as bass
import concourse.mybir as mybir
import concourse.tile as tile
from concourse.bass_utils import run_bass_kernel_spmd

FP = mybir.dt.float32
FPR = mybir.dt.float32r
U32 = mybir.dt.uint32
I16 = mybir.dt.int16
AX = mybir.AxisListType
ALU = mybir.AluOpType
ACTF = mybir.ActivationFunctionType

N_CORES = 8
K = 30
KPAD = 32          # 4 rounds of top-8
NEG = 0.2          # LeakyReLU slope
CH = 512           # distance chunk (columns per PSUM bank)
BSET = 4           # query blocks processed per chunk-sweep


def build_program(N=16384, E=262144, n_cores=N_CORES):
    NQ = N // n_cores            # queries per core
    NB = NQ // 128               # 128-query blocks per core
    ch = min(CH, NQ)             # distance chunk columns
    NCH = N // ch                # chunks per distance row
    NCAND = NCH * 8              # candidates per row
    EC = E // n_cores            # edges per core
    ECH = min(2048, EC)          # edges per gather chunk
    NECH = EC // ECH
    NSET = max(1, NB // BSET)
    BS = NB // NSET              # blocks per set
    assert NQ % 128 == 0 and N % ch == 0 and EC % ECH == 0 and ECH % 512 == 0
    assert NB % NSET == 0

    nc = bacc.Bacc("TRN2", target_bir_lowering=False, num_devices=n_cores)

    # ---------------- I/O ----------------
    m1_d = nc.dram_tensor("m1", [4, N], FP, kind="ExternalInput")
    stat1_d = nc.dram_tensor("stat1", [4, NQ], FP, kind="ExternalInput")
    wd1_d = nc.dram_tensor("wd1", [3, 64], FP, kind="ExternalInput")
    wb1_d = nc.dram_tensor("wb1", [3, 64], FP, kind="ExternalInput")
    b1_d = nc.dram_tensor("b1t", [64, 1], FP, kind="ExternalInput")
    wd2_d = nc.dram_tensor("wd2", [64, 64], FP, kind="ExternalInput")
    wb2_d = nc.dram_tensor("wb2", [64, 64], FP, kind="ExternalInput")
    b2_d = nc.dram_tensor("b2t", [64, 1], FP, kind="ExternalInput")
    wd3_d = nc.dram_tensor("wd3", [64, 64], FP, kind="ExternalInput")
    wb3_d = nc.dram_tensor("wb3", [64, 64], FP, kind="ExternalInput")
    b3_d = nc.dram_tensor("b3t", [64, 1], FP, kind="ExternalInput")
    wha_d = nc.dram_tensor("wha", [64, 64], FP, kind="ExternalInput")
    whb_d = nc.dram_tensor("whb", [64, 64], FP, kind="ExternalInput")
    bh1_d = nc.dram_tensor("bh1t", [64, 1], FP, kind="ExternalInput")
    w2_d = nc.dram_tensor("w2t", [64, 1], FP, kind="ExternalInput")
    bh2_d = nc.dram_tensor("bh2t", [1, 1], FP, kind="ExternalInput")
    ew0_d = nc.dram_tensor("ew0", [64, EC // 16], I16, kind="ExternalInput")
    ew1_d = nc.dram_tensor("ew1", [64, EC // 16], I16, kind="ExternalInput")
    out_d = nc.dram_tensor("out", [EC], FP, kind="ExternalOutput")

    with tile.TileContext(nc, num_cores=n_cores) as tc:
        with (
            tc.tile_pool(name="const", bufs=1) as cpool,
            tc.tile_pool(name="mchunk", bufs=4) as mcpool,
            tc.tile_pool(name="prep", bufs=3) as ppool,
            tc.tile_pool(name="xpool", bufs=1) as xpool,
            tc.tile_pool(name="upool", bufs=1) as upool,
            tc.tile_pool(name="cpoolc", bufs=2) as candpool,
            tc.tile_pool(name="pk", bufs=3) as pkpool,
            tc.tile_pool(name="small", bufs=3) as spool,
            tc.tile_pool(name="gat", bufs=4) as gpool,
            tc.tile_pool(name="dram", bufs=2, space="DRAM") as dpool,
            tc.tile_pool(name="pdist", bufs=3, space="PSUM") as pdist,
            tc.tile_pool(name="pmid", bufs=4, space="PSUM") as pmid,
        ):
            # ------------ load constants ------------
            def load(dram, shape, dtype=FP):
                t = cpool.tile(shape, dtype, tag=f"c_{dram.name}")
                nc.sync.dma_start(t[:], dram[:])
                return t

            stat1t = load(stat1_d, [4, NQ])
            wd = [load(wd1_d, [3, 64]), load(wd2_d, [64, 64]), load(wd3_d, [64, 64])]
            wb = [load(wb1_d, [3, 64]), load(wb2_d, [64, 64]), load(wb3_d, [64, 64])]
            bt = [load(b1_d, [64, 1]), load(b2_d, [64, 1]), load(b3_d, [64, 1])]
            what = load(wha_d, [64, 64])
            whbt = load(whb_d, [64, 64])
            bh1t = load(bh1_d, [64, 1])
            w2t = load(w2_d, [64, 1])
            bh2t = load(bh2_d, [1, 1])
            ew0t = load(ew0_d, [64, EC // 16], I16)
            ew1t = load(ew1_d, [64, EC // 16], I16)

            iota9 = cpool.tile([128, ch], U32)
            nc.gpsimd.iota(iota9[:], pattern=[[1, ch]], base=0, channel_multiplier=0)
            mask_hi = cpool.tile([128, 1], U32)
            nc.gpsimd.memset(mask_hi[:], 0xFFFFFE00)
            mask_lo = cpool.tile([128, 1], U32)
            nc.gpsimd.memset(mask_lo[:], 0x1FF)
            mask_p8 = cpool.tile([128, 1], U32)
            nc.gpsimd.memset(mask_p8[:], 0xFFFFFFF8)
            ones64 = cpool.tile([64, 1], FP)
            nc.gpsimd.memset(ones64[:], 1.0)

            cur_STAT = None   # [65, NQ] rows 0..63 2*x^T(local), row 64 ones
            cur_ccout = None  # DRAM [8*64, NQ], previous layer's AllGather
            XLT = None

            def conv_layer(li):
                nonlocal cur_STAT, cur_ccout, XLT
                Cin = 3 if li == 0 else 64
                KD = Cin + 1
                Ssrc = stat1t if li == 0 else cur_STAT

                # ---- pre-pass: u^T = wb^T.x^T [64, N]; (l>=2) -|x|^2 row ----
                UT = upool.tile([64, N], FP, tag="ut")
                sqrow = None
                if li > 0:
                    sqrow = dpool.tile([1, N], FP, tag="sqr")
                for c in range(NCH):
                    cs = slice(c * ch, (c + 1) * ch)
                    if li == 0:
                        trows = ppool.tile([3, ch], FP, tag="trows")
                        nc.sync.dma_start(trows[:], m1_d[0:3, cs])
                    else:
                        trows = ppool.tile([64, ch], FP, tag="trows")
                        rk, lc = (c * ch) // NQ, (c * ch) % NQ
                        nc.sync.dma_start(
                            trows[:],
                            cur_ccout[rk * 64:(rk + 1) * 64, lc:lc + ch],
                        )
                    pu = pmid.tile([64, ch], FP, tag="pmid")
                    nc.tensor.matmul(
                        pu[:], wb[li][:], trows[0:Cin, :]
                    )
                    nc.scalar.copy(UT[:, cs], pu[:])
                    if li > 0:
                        sqc = ppool.tile([64, ch], FP, tag="sqc")
                        nc.scalar.square(sqc[:], trows[:])
                        po = pmid.tile([1, ch], FP, tag="pmid")
                        nc.tensor.matmul(
                            po[:], ones64[:], sqc[:]
                        )
                        sqsb = ppool.tile([1, ch], FP, tag="sqsb")
                        nc.scalar.mul(sqsb[:], po[:], -1.0)
                        nc.sync.dma_start(sqrow[0:1, cs], sqsb[:])

                XLT_new = xpool.tile([64, NQ], FP, tag="xlt")
                for st in range(NSET):
                    blocks = range(st * BS, (st + 1) * BS)
                    cand = candpool.tile([128, BS * NCAND], FP, tag="cand")
                    # ---- distance sweep: chunks outer, blocks inner ----
                    for c in range(NCH):
                        cs = slice(c * ch, (c + 1) * ch)
                        mch = mcpool.tile([KD, ch], FP, tag="mch")
                        if li == 0:
                            nc.sync.dma_start(mch[:], m1_d[:, cs])
                        else:
                            rk, lc = (c * ch) // NQ, (c * ch) % NQ
                            nc.sync.dma_start(
                                mch[0:64, :],
                                cur_ccout[rk * 64:(rk + 1) * 64, lc:lc + ch],
                            )
                            nc.sync.dma_start(mch[64:65, :], sqrow[0:1, cs])
                        for bi, b in enumerate(blocks):
                            bs_ = slice(b * 128, (b + 1) * 128)
                            pd = pdist.tile([128, ch], FP, tag="pd")
                            nc.tensor.matmul(
                                pd[:],
                                Ssrc[0:KD, bs_],
                                mch[:],
                            )
                            pk = pkpool.tile([128, ch], U32, tag="pk")
                            nc.vector.scalar_tensor_tensor(
                                pk[:],
                                pd[:].bitcast(U32),
                                mask_hi[:],
                                iota9[:],
                                op0=ALU.bitwise_and,
                                op1=ALU.bitwise_or,
                            )
                            nc.vector.max(
                                cand[:, bi * NCAND + c * 8: bi * NCAND + c * 8 + 8],
                                pk[:].bitcast(FP),
                            )

                    # ---- per block: select, decode, gather, aggregate ----
                    for bi, b in enumerate(blocks):
                        bs_ = slice(b * 128, (b + 1) * 128)
                        cb = cand[:, bi * NCAND:(bi + 1) * NCAND]
                        wv = spool.tile([128, KPAD], FP, tag="wv")
                        pos = spool.tile([128, KPAD], U32, tag="pos")
                        for r in range(4):
                            rs = slice(r * 8, (r + 1) * 8)
                            nc.vector.max(wv[:, rs], cb)
                            nc.vector.max_index(pos[:, rs], wv[:, rs], cb)
                            if r < 3:
                                nc.vector.match_replace(cb, wv[:, rs], cb, -3.0e38)

                        posm = spool.tile([128, KPAD], U32, tag="posm")
                        nc.vector.scalar_tensor_tensor(
                            posm[:], pos[:], mask_p8[:], pos[:],
                            op0=ALU.bitwise_and, op1=ALU.bypass,
                        )
                        posf = spool.tile([128, KPAD], FP, tag="posf")
                        nc.vector.tensor_copy(posf[:], posm[:])
                        lom = spool.tile([128, KPAD], U32, tag="lom")
                        nc.vector.scalar_tensor_tensor(
                            lom[:], wv[:].bitcast(U32), mask_lo[:],
                            wv[:].bitcast(U32),
                            op0=ALU.bitwise_and, op1=ALU.bypass,
                        )
                        lof = spool.tile([128, KPAD], FP, tag="lof")
                        nc.vector.tensor_copy(lof[:], lom[:])
                        idxf = spool.tile([128, KPAD], FP, tag="idxf")
                        nc.vector.scalar_tensor_tensor(
                            idxf[:], posf[:], float(ch // 8), lof[:],
                            op0=ALU.mult, op1=ALU.add,
                        )
                        # ranks 31/32 are outside the top-30: duplicate rank 1
                        nc.vector.tensor_copy(idxf[:, 30:31], idxf[:, 0:1])
                        nc.vector.tensor_copy(idxf[:, 31:32], idxf[:, 0:1])
                        idx16 = spool.tile([128, KPAD], I16, tag="idx16")
                        nc.vector.tensor_copy(idx16[:], idxf[:])

                        # rewrap for ap_gather via DRAM bounce:
                        # element i = p*32+k -> (partition i%16, col i//16)
                        sc = dpool.tile([128, KPAD], I16, tag="scidx")
                        nc.sync.dma_start(sc[:], idx16[:])
                        wrap = spool.tile([64, KPAD * 8], I16, tag="wrap")
                        src = sc[:].rearrange("p (kh q) -> q p kh", q=16)
                        for g in range(4):
                            dst = wrap[g * 16:(g + 1) * 16, :].rearrange(
                                "q (p kh) -> q p kh", kh=2
                            )
                            nc.sync.dma_start(dst, src)

                        mT = spool.tile([64, 128], FP, tag="mT")
                        for h in range(2):
                            gath = gpool.tile([64, 64 * KPAD], FP, tag="gath")
                            nc.gpsimd.ap_gather(
                                gath[:], UT[:],
                                wrap[:, h * 128:(h + 1) * 128],
                                channels=64, num_elems=N, d=1,
                                num_idxs=64 * KPAD,
                            )
                            nc.vector.tensor_reduce(
                                mT[:, h * 64:(h + 1) * 64],
                                gath[:].rearrange("c (p k) -> c p k", k=KPAD),
                                axis=AX.X, op=ALU.max,
                            )

                        pv = pmid.tile([64, 128], FP, tag="pmid")
                        nc.tensor.matmul(pv[:], wd[li][:], Ssrc[0:Cin, bs_])
                        vT = spool.tile([64, 128], FP, tag="vT")
                        nc.scalar.activation(
                            vT[:], pv[:], ACTF.Identity, bias=bt[li][:]
                        )
                        zT = spool.tile([64, 128], FP, tag="zT")
                        nc.vector.tensor_tensor(zT[:], vT[:], mT[:], op=ALU.add)
                        nc.vector.scalar_tensor_tensor(
                            XLT_new[:, bs_], zT[:], NEG, zT[:],
                            op0=ALU.mult, op1=ALU.max,
                        )
                XLT = XLT_new

                # ---- AllGather new features ----
                ccin = dpool.tile([64, NQ], FP, tag="ccin")
                nc.sync.dma_start(ccin[:], XLT[:])
                ccout = dpool.tile(
                    [n_cores * 64, NQ], FP, tag="ccout", addr_space="Shared"
                )
                nc.gpsimd.collective_compute(
                    "AllGather",
                    ALU.bypass,
                    replica_groups=[list(range(n_cores))],
                    ins=[ccin[:].opt()],
                    outs=[ccout[:].opt()],
                )
                cur_ccout = ccout
                if li < 2:
                    STAT_new = xpool.tile([65, NQ], FP, tag="stat")
                    nc.scalar.mul(STAT_new[0:64, :], XLT[:], 2.0)
                    nc.gpsimd.memset(STAT_new[64:65, :], 1.0)
                    cur_STAT = STAT_new

            for li in range(3):
                conv_layer(li)

            # ---------------- edge head ----------------
            # x3^T full [64, N] from the final AllGather
            X3T = upool.tile([64, N], FP, tag="ut")
            for r in range(n_cores):
                nc.sync.dma_start(
                    X3T[:, r * NQ:(r + 1) * NQ],
                    cur_ccout[r * 64:(r + 1) * 64, :],
                )
            for ec in range(NECH):
                iw = ECH // 16
                g0 = gpool.tile([64, ECH], FP, tag="gath")
                nc.gpsimd.ap_gather(
                    g0[:], X3T[:], ew0t[:, ec * iw:(ec + 1) * iw],
                    channels=64, num_elems=N, d=1, num_idxs=ECH,
                )
                g1 = gpool.tile([64, ECH], FP, tag="gath")
                nc.gpsimd.ap_gather(
                    g1[:], X3T[:], ew1t[:, ec * iw:(ec + 1) * iw],
                    channels=64, num_elems=N, d=1, num_idxs=ECH,
                )
                for s in range(ECH // 512):
                    ss = slice(s * 512, (s + 1) * 512)
                    pz = pmid.tile([64, 512], FP, tag="pmid")
                    nc.tensor.matmul(
                        pz[:], what[:], g0[:, ss],
                        start=True, stop=False,
                    )
                    nc.tensor.matmul(
                        pz[:], whbt[:], g1[:, ss],
                        start=False, stop=True,
                    )
                    hE = spool.tile([64, 512], FP, tag="hE")
                    nc.scalar.activation(hE[:], pz[:], ACTF.Identity, bias=bh1t[:])
                    nc.vector.scalar_tensor_tensor(
                        hE[:], hE[:], NEG, hE[:], op0=ALU.mult, op1=ALU.max
                    )
                    po = pmid.tile([1, 512], FP, tag="pmid")
                    nc.tensor.matmul(
                        po[:], w2t[:], hE[:]
                    )
                    o512 = spool.tile([1, 512], FP, tag="o512")
                    nc.scalar.activation(o512[:], po[:], ACTF.Sigmoid, bias=bh2t[:])
                    nc.sync.dma_start(
                        out_d[ec * ECH + s * 512: ec * ECH + (s + 1) * 512],
                        o512[:],
                    )

    nc.compile()
    return nc


# ------------------------------------------------------------------
# host side
# ------------------------------------------------------------------

def prepare_inputs(x, edge_index, W1, b1, W2, b2, W3, b3, Wh1, bh1, Wh2, bh2,
                   n_cores=N_CORES):
    """Build the per-core input maps (all numpy, fp32)."""
    x = np.asarray(x, np.float32)
    N = x.shape[0]
    ei = np.asarray(edge_index)
    E = ei.shape[1]
    NQ = N // n_cores
    EC = E // n_cores
    ECH = min(2048, EC)

    xT = np.ascontiguousarray(x.T)                       # [3, N]
    sq = (x * x).sum(axis=1, dtype=np.float32)           # [N]
    m1 = np.concatenate([xT, -sq[None, :]], axis=0).astype(np.float32)

    def halfsplit(W, C):
        W = np.asarray(W, np.float32)
        return (0.5 * (W[:C] - W[C:])).astype(np.float32), np.ascontiguousarray(W[C:])

    wd1, wb1 = halfsplit(W1, 3)
    wd2, wb2 = halfsplit(W2, 64)
    wd3, wb3 = halfsplit(W3, 64)
    Wh1 = np.asarray(Wh1, np.float32)
    wha, whb = np.ascontiguousarray(Wh1[:64]), np.ascontiguousarray(Wh1[64:])

    v = np.sort(ei, axis=0)                              # canonical edges
    v0 = v[0].astype(np.int64)
    v1 = v[1].astype(np.int64)

    def wrap_idx(vals):
        # vals [EC] -> [64, EC//16] int16 wrapped per-16 within each
        # ECH-chunk, replicated across the 4 active Q7 core groups.
        segs = []
        for c in range(EC // ECH):
            seg = vals[c * ECH:(c + 1) * ECH].reshape(ECH // 16, 16).T
            segs.append(seg)
        w16 = np.concatenate(segs, axis=1).astype(np.int16)
        return np.tile(w16, (4, 1))

    common = {
        "m1": m1,
        "wd1": wd1, "wb1": wb1, "b1t": np.asarray(b1, np.float32).reshape(64, 1),
        "wd2": wd2, "wb2": wb2, "b2t": np.asarray(b2, np.float32).reshape(64, 1),
        "wd3": wd3, "wb3": wb3, "b3t": np.asarray(b3, np.float32).reshape(64, 1),
        "wha": wha, "whb": whb,
        "bh1t": np.asarray(bh1, np.float32).reshape(64, 1),
        "w2t": np.asarray(Wh2, np.float32).reshape(64, 1),
        "bh2t": np.asarray(bh2, np.float32).reshape(1, 1),
    }
    in_maps = []
    for r in range(n_cores):
        im = dict(common)
        im["stat1"] = np.concatenate(
            [2.0 * xT[:, r * NQ:(r + 1) * NQ], np.ones((1, NQ), np.float32)], axis=0
        ).astype(np.float32)
        im["ew0"] = wrap_idx(v0[r * EC:(r + 1) * EC])
        im["ew1"] = wrap_idx(v1[r * EC:(r + 1) * EC])
        in_maps.append(im)
    return in_maps


_CACHE = {}


def _get_program(N, E):
    key = (N, E)
    if key not in _CACHE:
        _CACHE[key] = build_program(N=N, E=E)
    return _CACHE[key]


def kernel(x, edge_index, W1, b1, W2, b2, W3, b3, Wh1, bh1, Wh2, bh2):
    x = np.asarray(x, np.float32)
    ei = np.asarray(edge_index)
    N, E = x.shape[0], ei.shape[1]
    nc = _get_program(N, E)
    in_maps = prepare_inputs(x, ei, W1, b1, W2, b2, W3, b3, Wh1, bh1, Wh2, bh2)
    res = run_bass_kernel_spmd(nc, in_maps, list(range(N_CORES)))
    outs = [np.asarray(res.results[i]["out"], np.float32) for i in range(N_CORES)]
    return np.concatenate(outs)



# revision 11
# speedup vs baseline: 2.0144x; 2.0144x over previous
"""Trainium2 Bass kernel for nn_ClusterPrediction (DynamicEdgeConv x3 + edge head).

Math (reference):
  3x DynamicEdgeConv: kNN(k=30) in feature space, per-edge MLP on
  [x_i, x_j - x_i] with LeakyReLU(0.2), max aggregation.
  Edge head on canonicalized (sorted) edge_index columns:
  sigmoid(W2 . LRelu(Wh1 . [x_u, x_v] + bh1) + bh2).

Device-side design (v2, restructured for overlap):
  * Distance ranking key: S_ij = 2 x_i.x_j - |x_j|^2 computed as one
    float32r matmul (1 cycle/row vs fp32's 4) with augmented contraction:
    stationary [2*x_q ; 1], moving [x_all ; -|x|^2], all SBUF-resident.
  * Sweep loops blocks OUTER / chunks INNER; per 512-col chunk: DVE packs
    the chunk-local index into the low 9 mantissa bits of the fp32 key,
    then DVE max -> per-chunk top-8 candidates.
  * Per-block tail: 4 rounds of max/max_index/match_replace -> ranks 1..32;
    index decode; the ap_gather index layout is produced ON-CHIP via PE
    transpose + strided DVE copies (no DRAM bounce).
  * Neighbor aggregation: u = Wb.x stacked twice -> [128, N] table; ONE
    128-channel ap_gather per block engages all 8 Q7 cores (queries 0-63 on
    cores 0-3, 64-127 on cores 4-7); DVE max-reduce; the two partition
    halves are folded back and summed with v_i INSIDE PSUM via two
    identity-slice matmuls.
  * Edge head: x3 stacked twice -> [128, N]; one 128-channel ap_gather per
    2048-edge chunk produces [x_u ; x_v] directly; single K=128 float32r
    matmul against the full Wh1.
  * Features sharded over 8 cores (2048 queries each); AllGather between
    layers.
"""

import numpy as np

import concourse.bacc as bacc
import concourse.bass as bass
import concourse.mybir as mybir
import concourse.tile as tile
from concourse.bass_utils import run_bass_kernel_spmd

FP = mybir.dt.float32
FPR = mybir.dt.float32r
U32 = mybir.dt.uint32
I16 = mybir.dt.int16
AX = mybir.AxisListType
ALU = mybir.AluOpType
ACTF = mybir.ActivationFunctionType

N_CORES = 8
K = 30
KPAD = 32          # 4 rounds of top-8
NEG = 0.2          # LeakyReLU slope
CH = 512           # distance chunk (columns per PSUM bank)


def build_program(N=16384, E=262144, n_cores=N_CORES):
    NQ = N // n_cores            # queries per core
    NB = NQ // 128               # 128-query blocks per core
    ch = CH
    NCH = N // ch                # chunks per distance row
    NCAND = NCH * 8              # candidates per row
    EC = E // n_cores            # edges per core
    ECH = 2048                   # edges per gather chunk
    NECH = EC // ECH
    assert NQ % 128 == 0 and N % ch == 0 and EC % ECH == 0

    nc = bacc.Bacc("TRN2", target_bir_lowering=False, num_devices=n_cores)

    # ---------------- I/O ----------------
    m1_d = nc.dram_tensor("m1", [4, N], FP, kind="ExternalInput")
    stat1_d = nc.dram_tensor("stat1", [4, NQ], FP, kind="ExternalInput")
    wd1_d = nc.dram_tensor("wd1", [3, 64], FP, kind="ExternalInput")
    wb1_d = nc.dram_tensor("wb1", [3, 64], FP, kind="ExternalInput")
    b1_d = nc.dram_tensor("b1t", [64, 1], FP, kind="ExternalInput")
    wd2_d = nc.dram_tensor("wd2", [64, 64], FP, kind="ExternalInput")
    wb2_d = nc.dram_tensor("wb2", [64, 64], FP, kind="ExternalInput")
    b2_d = nc.dram_tensor("b2t", [64, 1], FP, kind="ExternalInput")
    wd3_d = nc.dram_tensor("wd3", [64, 64], FP, kind="ExternalInput")
    wb3_d = nc.dram_tensor("wb3", [64, 64], FP, kind="ExternalInput")
    b3_d = nc.dram_tensor("b3t", [64, 1], FP, kind="ExternalInput")
    whf_d = nc.dram_tensor("whf", [128, 64], FP, kind="ExternalInput")
    bh1_d = nc.dram_tensor("bh1t", [64, 1], FP, kind="ExternalInput")
    w2_d = nc.dram_tensor("w2t", [64, 1], FP, kind="ExternalInput")
    bh2_d = nc.dram_tensor("bh2t", [1, 1], FP, kind="ExternalInput")
    ident_d = nc.dram_tensor("ident", [128, 128], FP, kind="ExternalInput")
    ew_d = nc.dram_tensor("ew", [128, EC // 16], I16, kind="ExternalInput")
    out_d = nc.dram_tensor("out", [EC], FP, kind="ExternalOutput")

    with tile.TileContext(nc, num_cores=n_cores) as tc:
        with (
            tc.tile_pool(name="const", bufs=1) as cpool,
            tc.tile_pool(name="mt", bufs=1) as mtpool,
            tc.tile_pool(name="u2", bufs=1) as u2pool,
            tc.tile_pool(name="prep", bufs=2) as ppool,
            tc.tile_pool(name="candp", bufs=2) as candpool,
            tc.tile_pool(name="pk", bufs=3) as pkpool,
            tc.tile_pool(name="small", bufs=3) as spool,
            tc.tile_pool(name="headp", bufs=2) as hpool,
            tc.tile_pool(name="wrapp", bufs=3) as wpool,
            tc.tile_pool(name="gat", bufs=2) as gpool,
            tc.tile_pool(name="dram", bufs=2, space="DRAM") as dpool,
            tc.tile_pool(name="pdist", bufs=3, space="PSUM") as pdist,
            tc.tile_pool(name="pmid", bufs=1, space="PSUM") as pmid,
            tc.tile_pool(name="ppv", bufs=1, space="PSUM") as ppv,
            tc.tile_pool(name="pvm", bufs=2, space="PSUM") as pvm,
        ):
            # ------------ load constants ------------
            def load(dram, shape, dtype=FP, eng=nc.sync):
                t = cpool.tile(shape, dtype, tag=f"c_{dram.name}")
                eng.dma_start(t[:], dram[:])
                return t

            stat1t = load(stat1_d, [4, NQ])
            wd = [load(wd1_d, [3, 64]), load(wd2_d, [64, 64]), load(wd3_d, [64, 64])]
            wb = [load(wb1_d, [3, 64]), load(wb2_d, [64, 64]), load(wb3_d, [64, 64])]
            bt = [load(b1_d, [64, 1]), load(b2_d, [64, 1]), load(b3_d, [64, 1])]
            whft = load(whf_d, [128, 64])
            bh1t = load(bh1_d, [64, 1])
            w2t = load(w2_d, [64, 1])
            bh2t = load(bh2_d, [1, 1])
            identt = load(ident_d, [128, 128], FP, eng=nc.scalar)
            ewt = load(ew_d, [128, EC // 16], I16, eng=nc.scalar)

            iota9 = cpool.tile([128, ch], U32)
            nc.gpsimd.iota(iota9[:], pattern=[[1, ch]], base=0, channel_multiplier=0)
            mask_hi = cpool.tile([128, 1], U32)
            nc.gpsimd.memset(mask_hi[:], 0xFFFFFE00)
            mask_lo = cpool.tile([128, 1], U32)
            nc.gpsimd.memset(mask_lo[:], 0x1FF)
            mask_p8 = cpool.tile([128, 1], U32)
            nc.gpsimd.memset(mask_p8[:], 0xFFFFFFF8)
            negones = cpool.tile([64, 1], FP)
            nc.gpsimd.memset(negones[:], -1.0)

            # big SBUF-resident tables
            MT = mtpool.tile([65, N], FP, tag="mt")      # moving operand
            U2 = u2pool.tile([128, N], FP, tag="u2")     # stacked gather table
            # STAT [65, NQ]: stationary (2*x^T; ones); row 64 preset once
            STAT = cpool.tile([65, NQ], FP, tag="stat")
            nc.gpsimd.memset(STAT[64:65, :], 1.0)
            XLT = cpool.tile([64, NQ], FP, tag="xlt")    # layer output

            # layer 1 moving operand: [x^T(3) ; -|x|^2] in MT rows 0..3
            nc.sync.dma_start(MT[0:4, :], m1_d[:])

            def conv_layer(li):
                Cin = 3 if li == 0 else 64
                KD = Cin + 1
                Ssrc = stat1t if li == 0 else STAT

                # ---- u table: U2[0:64] = wb^T . x^T ; duplicated to 64:128
                for c in range(NCH):
                    cs = slice(c * ch, (c + 1) * ch)
                    pu = pmid.tile([64, ch], FP, tag="pmid")
                    nc.tensor.matmul(
                        pu[:], wb[li][:],
                        MT[0:Cin, cs],
                    )
                    nc.scalar.copy(U2[0:64, cs], pu[:])
                nc.sync.dma_start(U2[64:128, :], U2[0:64, :])

                # per-block state carried between pipeline stages
                st = [dict() for _ in range(NB)]

                def sweep(b):
                    bs_ = slice(b * 128, (b + 1) * 128)
                    cand = candpool.tile([128, NCAND], FP, tag="cand")
                    st[b]["cand"] = cand
                    # v part: pv = wd . x_i  (0.5*(Wa-Wb) folded; Ssrc = 2x)
                    pv = ppv.tile([64, 128], FP, tag="ppv")
                    nc.tensor.matmul(
                        pv[:], wd[li][:],
                        Ssrc[0:Cin, bs_],
                    )
                    for c in range(NCH):
                        cs = slice(c * ch, (c + 1) * ch)
                        pd = pdist.tile([128, ch], FP, tag="pd")
                        nc.tensor.matmul(
                            pd[:],
                            Ssrc[0:KD, bs_],
                            MT[0:KD, cs],
                        )
                        pk = pkpool.tile([128, ch], U32, tag="pk")
                        nc.vector.scalar_tensor_tensor(
                            pk[:],
                            pd[:].bitcast(U32),
                            mask_hi[:],
                            iota9[:],
                            op0=ALU.bitwise_and,
                            op1=ALU.bitwise_or,
                        )
                        nc.vector.max(
                            cand[:, c * 8: c * 8 + 8],
                            pk[:].bitcast(FP),
                        )
                    # bias the v part right away (frees the PSUM buf early)
                    vT = spool.tile([64, 128], FP, tag="vT")
                    st[b]["vT"] = vT
                    nc.scalar.activation(
                        vT[:], pv[:], ACTF.Identity, bias=bt[li][:]
                    )

                def tail_a(b):
                    # top-32 select, index decode, rewrap, gather launch
                    cand = st[b].pop("cand")
                    wv = spool.tile([128, KPAD], FP, tag="wv")
                    pos = spool.tile([128, KPAD], U32, tag="pos")
                    for r in range(4):
                        rs = slice(r * 8, (r + 1) * 8)
                        nc.vector.max(wv[:, rs], cand[:])
                        nc.vector.max_index(pos[:, rs], wv[:, rs], cand[:])
                        if r < 3:
                            nc.vector.match_replace(
                                cand[:], wv[:, rs], cand[:], -3.0e38
                            )
                    posm = spool.tile([128, KPAD], U32, tag="posm")
                    nc.vector.scalar_tensor_tensor(
                        posm[:], pos[:], mask_p8[:], pos[:],
                        op0=ALU.bitwise_and, op1=ALU.bypass,
                    )
                    posf = spool.tile([128, KPAD], FP, tag="posf")
                    nc.vector.tensor_copy(posf[:], posm[:])
                    lom = spool.tile([128, KPAD], U32, tag="lom")
                    nc.vector.scalar_tensor_tensor(
                        lom[:], wv[:].bitcast(U32), mask_lo[:],
                        wv[:].bitcast(U32),
                        op0=ALU.bitwise_and, op1=ALU.bypass,
                    )
                    lof = spool.tile([128, KPAD], FP, tag="lof")
                    nc.vector.tensor_copy(lof[:], lom[:])
                    idxf = spool.tile([128, KPAD], FP, tag="idxf")
                    nc.vector.scalar_tensor_tensor(
                        idxf[:], posf[:], float(ch // 8), lof[:],
                        op0=ALU.mult, op1=ALU.add,
                    )
                    # ranks 31/32 are outside the top-30: duplicate rank 1
                    nc.vector.tensor_copy(idxf[:, 30:31], idxf[:, 0:1])
                    nc.vector.tensor_copy(idxf[:, 31:32], idxf[:, 0:1])

                    # on-chip rewrap to the ap_gather index layout:
                    # wrap[16g+p, 2q+h] = idx[q + 64*(g>=4)][16h+p]
                    wrap = wpool.tile([128, 128], I16, tag="wrap")
                    for h in range(2):
                        rep = wpool.tile([128, 128], FP, tag=f"rep{h}")
                        nc.vector.tensor_copy(
                            rep[:, 0:16], idxf[:, 16 * h:16 * h + 16]
                        )
                        nc.vector.tensor_copy(rep[:, 16:32], rep[:, 0:16])
                        nc.vector.tensor_copy(rep[:, 32:64], rep[:, 0:32])
                        nc.vector.tensor_copy(rep[:, 64:128], rep[:, 0:64])
                        tp = pdist.tile([128, 128], FP, tag="pd")
                        nc.tensor.transpose(tp[:], rep[:], identt[:])
                        # cores 0-3 <- queries 0-63; cores 4-7 <- 64-127
                        nc.vector.tensor_copy(
                            wrap[0:64, :].rearrange("p (q t) -> p q t", t=2)[
                                :, :, h
                            ],
                            tp[0:64, 0:64],
                        )
                        nc.vector.tensor_copy(
                            wrap[64:128, :].rearrange("p (q t) -> p q t", t=2)[
                                :, :, h
                            ],
                            tp[64:128, 64:128],
                        )

                    # gather u columns for all 128 queries on all 8 Q7 cores
                    gath = gpool.tile([128, 2048], FP, tag="gath")
                    st[b]["gath"] = gath
                    nc.gpsimd.ap_gather(
                        gath[:], U2[:], wrap[:],
                        channels=128, num_elems=N, d=1,
                        num_idxs=2048,
                    )
                def tail_b(b):
                    bs_ = slice(b * 128, (b + 1) * 128)
                    gath = st[b].pop("gath")
                    mx = spool.tile([128, 64], FP, tag="mx")
                    nc.vector.tensor_reduce(
                        mx[:],
                        gath[:].rearrange("c (p k) -> c p k", k=KPAD),
                        axis=AX.X, op=ALU.max,
                    )
                    # fold the two partition halves of mx back to [64, 128]
                    pm = pvm.tile([64, 128], FP, tag="pvm")
                    nc.tensor.matmul(
                        pm[:, 0:64], identt[:, 0:64],
                        mx[:],
                        start=True, stop=False,
                    )
                    nc.tensor.matmul(
                        pm[:, 64:128], identt[:, 64:128],
                        mx[:],
                        start=False, stop=True,
                    )
                    vT = st[b].pop("vT")
                    zT = spool.tile([64, 128], FP, tag="zT")
                    nc.vector.tensor_tensor(zT[:], vT[:], pm[:], op=ALU.add)
                    nc.vector.scalar_tensor_tensor(
                        XLT[:, bs_], zT[:], NEG, zT[:],
                        op0=ALU.mult, op1=ALU.max,
                    )

                for it in range(NB + 2):
                    if it < NB:
                        sweep(it)
                    if 1 <= it <= NB:
                        tail_a(it - 1)
                    if it >= 2:
                        tail_b(it - 2)

                # ---- AllGather new features ----
                ccin = dpool.tile([64, NQ], FP, tag="ccin")
                nc.sync.dma_start(ccin[:], XLT[:])
                ccout = dpool.tile(
                    [n_cores * 64, NQ], FP, tag="ccout", addr_space="Shared"
                )
                nc.gpsimd.collective_compute(
                    "AllGather",
                    ALU.bypass,
                    replica_groups=[list(range(n_cores))],
                    ins=[ccin[:].opt()],
                    outs=[ccout[:].opt()],
                )

                # ---- rebuild MT / STAT for next layer (or U2 for head) ----
                if li < 2:
                    for r in range(n_cores):
                        nc.scalar.dma_start(
                            MT[0:64, r * NQ:(r + 1) * NQ],
                            ccout[r * 64:(r + 1) * 64, :],
                        )
                    nc.scalar.mul(STAT[0:64, :], XLT[:], 2.0)
                    for c in range(NCH):
                        cs = slice(c * ch, (c + 1) * ch)
                        sqc = ppool.tile([64, ch], FP, tag="sqc")
                        nc.scalar.square(sqc[:], MT[0:64, cs])
                        po = pmid.tile([1, ch], FP, tag="pmid")
                        nc.tensor.matmul(
                            po[:], negones[:],
                            sqc[:],
                        )
                        nc.scalar.copy(MT[64:65, cs], po[:])
                else:
                    for r in range(n_cores):
                        nc.scalar.dma_start(
                            U2[0:64, r * NQ:(r + 1) * NQ],
                            ccout[r * 64:(r + 1) * 64, :],
                        )
                    nc.sync.dma_start(U2[64:128, :], U2[0:64, :])

            for li in range(3):
                conv_layer(li)

            # ---------------- edge head ----------------
            iw = ECH // 16
            for ec in range(NECH):
                g0 = gpool.tile([128, ECH], FP, tag="gath")
                nc.gpsimd.ap_gather(
                    g0[:], U2[:], ewt[:, ec * iw:(ec + 1) * iw],
                    channels=128, num_elems=N, d=1, num_idxs=ECH,
                )
                for s in range(ECH // 512):
                    ss = slice(s * 512, (s + 1) * 512)
                    pz = pmid.tile([64, 512], FP, tag="pmid")
                    nc.tensor.matmul(
                        pz[:], whft[:], g0[:, ss]
                    )
                    hE = hpool.tile([64, 512], FP, tag="hE")
                    nc.scalar.activation(hE[:], pz[:], ACTF.Identity, bias=bh1t[:])
                    nc.vector.scalar_tensor_tensor(
                        hE[:], hE[:], NEG, hE[:], op0=ALU.mult, op1=ALU.max
                    )
                    po = pmid.tile([1, 512], FP, tag="ppo")
                    nc.tensor.matmul(
                        po[:], w2t[:], hE[:]
                    )
                    o512 = hpool.tile([1, 512], FP, tag="o512")
                    nc.scalar.activation(
                        o512[:], po[:], ACTF.Sigmoid, bias=bh2t[:]
                    )
                    nc.sync.dma_start(
                        out_d[ec * ECH + s * 512: ec * ECH + (s + 1) * 512],
                        o512[:],
                    )

    nc.compile()
    return nc


# ------------------------------------------------------------------
# host side
# ------------------------------------------------------------------

def prepare_inputs(x, edge_index, W1, b1, W2, b2, W3, b3, Wh1, bh1, Wh2, bh2,
                   n_cores=N_CORES):
    """Build the per-core input maps (all numpy, fp32)."""
    x = np.asarray(x, np.float32)
    N = x.shape[0]
    ei = np.asarray(edge_index)
    E = ei.shape[1]
    NQ = N // n_cores
    EC = E // n_cores
    ECH = 2048

    xT = np.ascontiguousarray(x.T)                       # [3, N]
    sq = (x * x).sum(axis=1, dtype=np.float32)           # [N]
    m1 = np.concatenate([xT, -sq[None, :]], axis=0).astype(np.float32)

    def halfsplit(W, C):
        W = np.asarray(W, np.float32)
        return (0.5 * (W[:C] - W[C:])).astype(np.float32), np.ascontiguousarray(W[C:])

    wd1, wb1 = halfsplit(W1, 3)
    wd2, wb2 = halfsplit(W2, 64)
    wd3, wb3 = halfsplit(W3, 64)

    v = np.sort(ei, axis=0)                              # canonical edges
    v0 = v[0].astype(np.int64)
    v1 = v[1].astype(np.int64)

    def wrap_head(v0r, v1r):
        # [128, EC//16] int16: per 2048-edge chunk, cores 0-3 get the v0
        # list, cores 4-7 the v1 list, each wrapped %16 over partitions.
        ew = np.zeros((128, EC // 16), np.int16)
        for ec in range(EC // ECH):
            s0 = v0r[ec * ECH:(ec + 1) * ECH].reshape(ECH // 16, 16).T
            s1 = v1r[ec * ECH:(ec + 1) * ECH].reshape(ECH // 16, 16).T
            cs = slice(ec * (ECH // 16), (ec + 1) * (ECH // 16))
            for g in range(4):
                ew[16 * g:16 * g + 16, cs] = s0
            for g in range(4, 8):
                ew[16 * g:16 * g + 16, cs] = s1
        return ew

    common = {
        "m1": m1,
        "wd1": wd1, "wb1": wb1, "b1t": np.asarray(b1, np.float32).reshape(64, 1),
        "wd2": wd2, "wb2": wb2, "b2t": np.asarray(b2, np.float32).reshape(64, 1),
        "wd3": wd3, "wb3": wb3, "b3t": np.asarray(b3, np.float32).reshape(64, 1),
        "whf": np.ascontiguousarray(np.asarray(Wh1, np.float32)),
        "bh1t": np.asarray(bh1, np.float32).reshape(64, 1),
        "w2t": np.asarray(Wh2, np.float32).reshape(64, 1),
        "bh2t": np.asarray(bh2, np.float32).reshape(1, 1),
        "ident": np.eye(128, dtype=np.float32),
    }
    in_maps = []
    for r in range(n_cores):
        im = dict(common)
        im["stat1"] = np.concatenate(
            [2.0 * xT[:, r * NQ:(r + 1) * NQ], np.ones((1, NQ), np.float32)], axis=0
        ).astype(np.float32)
        im["ew"] = wrap_head(v0[r * EC:(r + 1) * EC], v1[r * EC:(r + 1) * EC])
        in_maps.append(im)
    return in_maps


_CACHE = {}


def _get_program(N, E):
    key = (N, E)
    if key not in _CACHE:
        _CACHE[key] = build_program(N=N, E=E)
    return _CACHE[key]


def kernel(x, edge_index, W1, b1, W2, b2, W3, b3, Wh1, bh1, Wh2, bh2):
    x = np.asarray(x, np.float32)
    ei = np.asarray(edge_index)
    N, E = x.shape[0], ei.shape[1]
    nc = _get_program(N, E)
    in_maps = prepare_inputs(x, ei, W1, b1, W2, b2, W3, b3, Wh1, bh1, Wh2, bh2)
    res = run_bass_kernel_spmd(nc, in_maps, list(range(N_CORES)))
    outs = [np.asarray(res.results[i]["out"], np.float32) for i in range(N_CORES)]
    return np.concatenate(outs)


# revision 15
# speedup vs baseline: 2.0315x; 1.0085x over previous
"""Trainium2 Bass kernel for nn_ClusterPrediction (DynamicEdgeConv x3 + edge head).

Math (reference):
  3x DynamicEdgeConv: kNN(k=30) in feature space, per-edge MLP on
  [x_i, x_j - x_i] with LeakyReLU(0.2), max aggregation.
  Edge head on canonicalized (sorted) edge_index columns:
  sigmoid(W2 . LRelu(Wh1 . [x_u, x_v] + bh1) + bh2).

Device-side design (v2, restructured for overlap):
  * Distance ranking key: S_ij = 2 x_i.x_j - |x_j|^2 computed as one
    float32r matmul (1 cycle/row vs fp32's 4) with augmented contraction:
    stationary [2*x_q ; 1], moving [x_all ; -|x|^2], all SBUF-resident.
  * Sweep loops blocks OUTER / chunks INNER; per 512-col chunk: DVE packs
    the chunk-local index into the low 9 mantissa bits of the fp32 key,
    then DVE max -> per-chunk top-8 candidates.
  * Per-block tail: 4 rounds of max/max_index/match_replace -> ranks 1..32;
    index decode; the ap_gather index layout is produced ON-CHIP via PE
    transpose + strided DVE copies (no DRAM bounce).
  * Neighbor aggregation: u = Wb.x stacked twice -> [128, N] table; ONE
    128-channel ap_gather per block engages all 8 Q7 cores (queries 0-63 on
    cores 0-3, 64-127 on cores 4-7); DVE max-reduce; the two partition
    halves are folded back and summed with v_i INSIDE PSUM via two
    identity-slice matmuls.
  * Edge head: x3 stacked twice -> [128, N]; one 128-channel ap_gather per
    2048-edge chunk produces [x_u ; x_v] directly; single K=128 float32r
    matmul against the full Wh1.
  * Features sharded over 8 cores (2048 queries each); AllGather between
    layers.
"""

import numpy as np

import concourse.bacc as bacc
import concourse.bass as bass
import concourse.mybir as mybir
import concourse.tile as tile
from concourse.bass_utils import run_bass_kernel_spmd

FP = mybir.dt.float32
FPR = mybir.dt.float32r
U32 = mybir.dt.uint32
I16 = mybir.dt.int16
AX = mybir.AxisListType
ALU = mybir.AluOpType
ACTF = mybir.ActivationFunctionType

N_CORES = 8
K = 30
KPAD = 32          # 4 rounds of top-8
NEG = 0.2          # LeakyReLU slope
CH = 512           # distance chunk (columns per PSUM bank)


def build_program(N=16384, E=262144, n_cores=N_CORES):
    NQ = N // n_cores            # queries per core
    NB = NQ // 128               # 128-query blocks per core
    ch = CH
    NCH = N // ch                # chunks per distance row
    NCAND = (NCH // 2) * 8       # candidates per row (top-8 per 1024)
    EC = E // n_cores            # edges per core
    ECH = 2048                   # edges per gather chunk
    NECH = EC // ECH
    assert NQ % 128 == 0 and N % ch == 0 and EC % ECH == 0

    nc = bacc.Bacc("TRN2", target_bir_lowering=False, num_devices=n_cores)

    # ---------------- I/O ----------------
    m1_d = nc.dram_tensor("m1", [4, N], FP, kind="ExternalInput")
    stat1_d = nc.dram_tensor("stat1", [4, NQ], FP, kind="ExternalInput")
    wd1_d = nc.dram_tensor("wd1", [3, 64], FP, kind="ExternalInput")
    wb1_d = nc.dram_tensor("wb1", [3, 64], FP, kind="ExternalInput")
    b1_d = nc.dram_tensor("b1t", [64, 1], FP, kind="ExternalInput")
    wd2_d = nc.dram_tensor("wd2", [64, 64], FP, kind="ExternalInput")
    wb2_d = nc.dram_tensor("wb2", [64, 64], FP, kind="ExternalInput")
    b2_d = nc.dram_tensor("b2t", [64, 1], FP, kind="ExternalInput")
    wd3_d = nc.dram_tensor("wd3", [64, 64], FP, kind="ExternalInput")
    wb3_d = nc.dram_tensor("wb3", [64, 64], FP, kind="ExternalInput")
    b3_d = nc.dram_tensor("b3t", [64, 1], FP, kind="ExternalInput")
    whf_d = nc.dram_tensor("whf", [128, 64], FP, kind="ExternalInput")
    bh1_d = nc.dram_tensor("bh1t", [64, 1], FP, kind="ExternalInput")
    w2_d = nc.dram_tensor("w2t", [64, 1], FP, kind="ExternalInput")
    bh2_d = nc.dram_tensor("bh2t", [1, 1], FP, kind="ExternalInput")
    ident_d = nc.dram_tensor("ident", [128, 128], FP, kind="ExternalInput")
    ew_d = nc.dram_tensor("ew", [128, EC // 16], I16, kind="ExternalInput")
    out_d = nc.dram_tensor("out", [EC], FP, kind="ExternalOutput")

    with tile.TileContext(nc, num_cores=n_cores) as tc:
        with (
            tc.tile_pool(name="const", bufs=1) as cpool,
            tc.tile_pool(name="mt", bufs=1) as mtpool,
            tc.tile_pool(name="u2", bufs=1) as u2pool,
            tc.tile_pool(name="prep", bufs=2) as ppool,
            tc.tile_pool(name="candp", bufs=2) as candpool,
            tc.tile_pool(name="pk", bufs=2) as pkpool,
            tc.tile_pool(name="small", bufs=3) as spool,
            tc.tile_pool(name="wrapp", bufs=3) as wpool,
            tc.tile_pool(name="gat", bufs=3) as gpool,
            tc.tile_pool(name="vtp", bufs=4) as vtpool,
            tc.tile_pool(name="dram", bufs=2, space="DRAM") as dpool,
            tc.tile_pool(name="pdist", bufs=3, space="PSUM") as pdist,
            tc.tile_pool(name="pmid", bufs=1, space="PSUM") as pmid,
            tc.tile_pool(name="ppv", bufs=1, space="PSUM") as ppv,
            tc.tile_pool(name="pvm", bufs=2, space="PSUM") as pvm,
        ):
            # ------------ load constants ------------
            def load(dram, shape, dtype=FP, eng=nc.sync):
                t = cpool.tile(shape, dtype, tag=f"c_{dram.name}")
                eng.dma_start(t[:], dram[:])
                return t

            wd = [load(wd1_d, [3, 64]), load(wd2_d, [64, 64]), load(wd3_d, [64, 64])]
            wb = [load(wb1_d, [3, 64]), load(wb2_d, [64, 64]), load(wb3_d, [64, 64])]
            bt = [load(b1_d, [64, 1]), load(b2_d, [64, 1]), load(b3_d, [64, 1])]
            whft = load(whf_d, [128, 64])
            bh1t = load(bh1_d, [64, 1])
            w2t = load(w2_d, [64, 1])
            bh2t = load(bh2_d, [1, 1])
            identt = load(ident_d, [128, 128], FP, eng=nc.scalar)
            ewt = load(ew_d, [128, EC // 16], I16, eng=nc.scalar)

            iota10 = cpool.tile([128, 2 * ch], U32)
            nc.gpsimd.iota(iota10[:], pattern=[[1, 2 * ch]], base=0, channel_multiplier=0)
            mask_hi = cpool.tile([128, 1], U32)
            nc.gpsimd.memset(mask_hi[:], 0xFFFFFC00)
            mask_lo = cpool.tile([128, 1], U32)
            nc.gpsimd.memset(mask_lo[:], 0x3FF)
            mask_p8 = cpool.tile([128, 1], U32)
            nc.gpsimd.memset(mask_p8[:], 0xFFFFFFF8)
            negones = cpool.tile([64, 1], FP)
            nc.gpsimd.memset(negones[:], -1.0)

            # big SBUF-resident tables
            MT = mtpool.tile([65, N], FP, tag="mt")      # moving operand
            U2 = u2pool.tile([128, N], FP, tag="u2")     # stacked gather table
            # STAT [65, NQ]: stationary (2*x^T; ones); row 64 preset once
            STAT = cpool.tile([65, NQ], FP, tag="stat")
            nc.gpsimd.memset(STAT[64:65, :], 1.0)
            nc.sync.dma_start(STAT[0:4, :], stat1_d[:])
            XLT = cpool.tile([64, NQ], FP, tag="xlt")    # layer output

            # layer 1 moving operand: [x^T(3) ; -|x|^2] in MT rows 0..3
            nc.sync.dma_start(MT[0:4, :], m1_d[:])

            def conv_layer(li):
                Cin = 3 if li == 0 else 64
                KD = Cin + 1
                Ssrc = STAT

                # ---- u table: U2[0:64] = wb^T . x^T ; duplicated to 64:128
                for c in range(NCH):
                    cs = slice(c * ch, (c + 1) * ch)
                    pu = pmid.tile([64, ch], FP, tag="pmid")
                    nc.tensor.matmul(
                        pu[:], wb[li][:],
                        MT[0:Cin, cs],
                    )
                    nc.scalar.copy(U2[0:64, cs], pu[:])
                nc.sync.dma_start(U2[64:128, :], U2[0:64, :])

                # per-block state carried between pipeline stages
                st = [dict() for _ in range(NB)]

                def sweep(b):
                    bs_ = slice(b * 128, (b + 1) * 128)
                    cand = candpool.tile([128, NCAND], FP, tag="cand")
                    st[b]["cand"] = cand
                    # v part: pv = wd . x_i  (0.5*(Wa-Wb) folded; Ssrc = 2x)
                    pv = ppv.tile([64, 128], FP, tag="ppv")
                    nc.tensor.matmul(
                        pv[:], wd[li][:],
                        Ssrc[0:Cin, bs_],
                    )
                    for g in range(NCH // 2):
                        pk = pkpool.tile([128, 2 * ch], U32, tag="pk")
                        for half in range(2):
                            c = 2 * g + half
                            cs = slice(c * ch, (c + 1) * ch)
                            hs = slice(half * ch, (half + 1) * ch)
                            pd = pdist.tile([128, ch], FP, tag="pd")
                            nc.tensor.matmul(
                                pd[:],
                                Ssrc[0:KD, bs_],
                                MT[0:KD, cs],
                            )
                            nc.vector.scalar_tensor_tensor(
                                pk[:, hs],
                                pd[:].bitcast(U32),
                                mask_hi[:],
                                iota10[:, hs],
                                op0=ALU.bitwise_and,
                                op1=ALU.bitwise_or,
                            )
                        nc.vector.max(
                            cand[:, g * 8: g * 8 + 8],
                            pk[:].bitcast(FP),
                        )
                    # bias the v part right away (frees the PSUM buf early)
                    vT = vtpool.tile([64, 128], FP, tag="vT")
                    st[b]["vT"] = vT
                    nc.scalar.activation(
                        vT[:], pv[:], ACTF.Identity, bias=bt[li][:]
                    )

                def tail_a(b):
                    # top-32 select, index decode, rewrap, gather launch
                    cand = st[b].pop("cand")
                    wv = spool.tile([128, KPAD], FP, tag="wv")
                    pos = spool.tile([128, KPAD], U32, tag="pos")
                    for r in range(4):
                        rs = slice(r * 8, (r + 1) * 8)
                        nc.vector.max(wv[:, rs], cand[:])
                        nc.vector.max_index(pos[:, rs], wv[:, rs], cand[:])
                        if r < 3:
                            nc.vector.match_replace(
                                cand[:], wv[:, rs], cand[:], -3.0e38
                            )
                    posm = spool.tile([128, KPAD], U32, tag="posm")
                    nc.vector.scalar_tensor_tensor(
                        posm[:], pos[:], mask_p8[:], pos[:],
                        op0=ALU.bitwise_and, op1=ALU.bypass,
                    )
                    posf = spool.tile([128, KPAD], FP, tag="posf")
                    nc.vector.tensor_copy(posf[:], posm[:])
                    lom = spool.tile([128, KPAD], U32, tag="lom")
                    nc.vector.scalar_tensor_tensor(
                        lom[:], wv[:].bitcast(U32), mask_lo[:],
                        wv[:].bitcast(U32),
                        op0=ALU.bitwise_and, op1=ALU.bypass,
                    )
                    lof = spool.tile([128, KPAD], FP, tag="lof")
                    nc.vector.tensor_copy(lof[:], lom[:])
                    idxf = spool.tile([128, KPAD], FP, tag="idxf")
                    nc.vector.scalar_tensor_tensor(
                        idxf[:], posf[:], float(2 * ch // 8), lof[:],
                        op0=ALU.mult, op1=ALU.add,
                    )
                    # ranks 31/32 are outside the top-30: duplicate rank 1
                    nc.scalar.copy(idxf[:, 30:31], idxf[:, 0:1])
                    nc.scalar.copy(idxf[:, 31:32], idxf[:, 0:1])

                    # on-chip rewrap to the ap_gather index layout:
                    # wrap[16g+p, 2q+h] = idx[q + 64*(g>=4)][16h+p]
                    wrap = wpool.tile([128, 128], I16, tag="wrap")
                    for h in range(2):
                        rep = wpool.tile([128, 128], FP, tag=f"rep{h}")
                        nc.scalar.copy(
                            rep[:, 0:16], idxf[:, 16 * h:16 * h + 16]
                        )
                        nc.scalar.copy(rep[:, 16:32], rep[:, 0:16])
                        nc.scalar.copy(rep[:, 32:64], rep[:, 0:32])
                        nc.scalar.copy(rep[:, 64:128], rep[:, 0:64])
                        tp = pdist.tile([128, 128], FP, tag="pd")
                        nc.tensor.transpose(tp[:], rep[:], identt[:])
                        # cores 0-3 <- queries 0-63; cores 4-7 <- 64-127
                        nc.vector.tensor_copy(
                            wrap[0:64, :].rearrange("p (q t) -> p q t", t=2)[
                                :, :, h
                            ],
                            tp[0:64, 0:64],
                        )
                        nc.vector.tensor_copy(
                            wrap[64:128, :].rearrange("p (q t) -> p q t", t=2)[
                                :, :, h
                            ],
                            tp[64:128, 64:128],
                        )

                    # gather u columns for all 128 queries on all 8 Q7 cores
                    gath = gpool.tile([128, 2048], FP, tag="gath")
                    st[b]["gath"] = gath
                    nc.gpsimd.ap_gather(
                        gath[:], U2[:], wrap[:],
                        channels=128, num_elems=N, d=1,
                        num_idxs=2048,
                    )
                def tail_b(b):
                    bs_ = slice(b * 128, (b + 1) * 128)
                    gath = st[b].pop("gath")
                    mx = spool.tile([128, 64], FP, tag="mx")
                    nc.vector.tensor_reduce(
                        mx[:],
                        gath[:].rearrange("c (p k) -> c p k", k=KPAD),
                        axis=AX.X, op=ALU.max,
                    )
                    # fold the two partition halves of mx back to [64, 128]
                    pm = pvm.tile([64, 128], FP, tag="pvm")
                    nc.tensor.matmul(
                        pm[:, 0:64], identt[:, 0:64],
                        mx[:],
                        start=True, stop=False,
                    )
                    nc.tensor.matmul(
                        pm[:, 64:128], identt[:, 64:128],
                        mx[:],
                        start=False, stop=True,
                    )
                    vT = st[b].pop("vT")
                    zT = spool.tile([64, 128], FP, tag="zT")
                    nc.vector.tensor_tensor(zT[:], vT[:], pm[:], op=ALU.add)
                    nc.vector.scalar_tensor_tensor(
                        XLT[:, bs_], zT[:], NEG, zT[:],
                        op0=ALU.mult, op1=ALU.max,
                    )

                for it in range(NB + 3):
                    if it < NB:
                        sweep(it)
                    if 1 <= it <= NB:
                        tail_a(it - 1)
                    if it >= 3:
                        tail_b(it - 3)

                # ---- AllGather new features ----
                ccin = dpool.tile([64, NQ], FP, tag="ccin")
                nc.sync.dma_start(ccin[:], XLT[:])
                ccout = dpool.tile(
                    [n_cores * 64, NQ], FP, tag="ccout", addr_space="Shared"
                )
                nc.gpsimd.collective_compute(
                    "AllGather",
                    ALU.bypass,
                    replica_groups=[list(range(n_cores))],
                    ins=[ccin[:].opt()],
                    outs=[ccout[:].opt()],
                )

                # ---- rebuild MT / STAT for next layer (or U2 for head) ----
                if li < 2:
                    for r in range(n_cores):
                        nc.scalar.dma_start(
                            MT[0:64, r * NQ:(r + 1) * NQ],
                            ccout[r * 64:(r + 1) * 64, :],
                        )
                    nc.scalar.mul(STAT[0:64, :], XLT[:], 2.0)
                    for c in range(NCH):
                        cs = slice(c * ch, (c + 1) * ch)
                        sqc = ppool.tile([64, ch], FP, tag="sqc")
                        nc.scalar.square(sqc[:], MT[0:64, cs])
                        po = pmid.tile([1, ch], FP, tag="pmid")
                        nc.tensor.matmul(
                            po[:], negones[:],
                            sqc[:],
                        )
                        nc.scalar.copy(MT[64:65, cs], po[:])
                else:
                    for r in range(n_cores):
                        nc.scalar.dma_start(
                            U2[0:64, r * NQ:(r + 1) * NQ],
                            ccout[r * 64:(r + 1) * 64, :],
                        )
                    nc.sync.dma_start(U2[64:128, :], U2[0:64, :])

            for li in range(3):
                conv_layer(li)

            # ---------------- edge head ----------------
            iw = ECH // 16
            for ec in range(NECH):
                g0 = gpool.tile([128, ECH], FP, tag="gath")
                nc.gpsimd.ap_gather(
                    g0[:], U2[:], ewt[:, ec * iw:(ec + 1) * iw],
                    channels=128, num_elems=N, d=1, num_idxs=ECH,
                )
                for s in range(ECH // 512):
                    ss = slice(s * 512, (s + 1) * 512)
                    pz = pmid.tile([64, 512], FP, tag="pmid")
                    nc.tensor.matmul(
                        pz[:], whft[:], g0[:, ss]
                    )
                    hE = pkpool.tile([64, 512], FP, tag="pk")
                    nc.scalar.activation(hE[:], pz[:], ACTF.Identity, bias=bh1t[:])
                    nc.vector.scalar_tensor_tensor(
                        hE[:], hE[:], NEG, hE[:], op0=ALU.mult, op1=ALU.max
                    )
                    po = pmid.tile([1, 512], FP, tag="ppo")
                    nc.tensor.matmul(
                        po[:], w2t[:], hE[:]
                    )
                    o512 = pkpool.tile([1, 512], FP, tag="pk")
                    nc.scalar.activation(
                        o512[:], po[:], ACTF.Sigmoid, bias=bh2t[:]
                    )
                    nc.sync.dma_start(
                        out_d[ec * ECH + s * 512: ec * ECH + (s + 1) * 512],
                        o512[:],
                    )

    nc.compile()
    return nc


# ------------------------------------------------------------------
# host side
# ------------------------------------------------------------------

def prepare_inputs(x, edge_index, W1, b1, W2, b2, W3, b3, Wh1, bh1, Wh2, bh2,
                   n_cores=N_CORES):
    """Build the per-core input maps (all numpy, fp32)."""
    x = np.asarray(x, np.float32)
    N = x.shape[0]
    ei = np.asarray(edge_index)
    E = ei.shape[1]
    NQ = N // n_cores
    EC = E // n_cores
    ECH = 2048

    xT = np.ascontiguousarray(x.T)                       # [3, N]
    sq = (x * x).sum(axis=1, dtype=np.float32)           # [N]
    m1 = np.concatenate([xT, -sq[None, :]], axis=0).astype(np.float32)

    def halfsplit(W, C):
        W = np.asarray(W, np.float32)
        return (0.5 * (W[:C] - W[C:])).astype(np.float32), np.ascontiguousarray(W[C:])

    wd1, wb1 = halfsplit(W1, 3)
    wd2, wb2 = halfsplit(W2, 64)
    wd3, wb3 = halfsplit(W3, 64)

    v = np.sort(ei, axis=0)                              # canonical edges
    v0 = v[0].astype(np.int64)
    v1 = v[1].astype(np.int64)

    def wrap_head(v0r, v1r):
        # [128, EC//16] int16: per 2048-edge chunk, cores 0-3 get the v0
        # list, cores 4-7 the v1 list, each wrapped %16 over partitions.
        ew = np.zeros((128, EC // 16), np.int16)
        for ec in range(EC // ECH):
            s0 = v0r[ec * ECH:(ec + 1) * ECH].reshape(ECH // 16, 16).T
            s1 = v1r[ec * ECH:(ec + 1) * ECH].reshape(ECH // 16, 16).T
            cs = slice(ec * (ECH // 16), (ec + 1) * (ECH // 16))
            for g in range(4):
                ew[16 * g:16 * g + 16, cs] = s0
            for g in range(4, 8):
                ew[16 * g:16 * g + 16, cs] = s1
        return ew

    common = {
        "m1": m1,
        "wd1": wd1, "wb1": wb1, "b1t": np.asarray(b1, np.float32).reshape(64, 1),
        "wd2": wd2, "wb2": wb2, "b2t": np.asarray(b2, np.float32).reshape(64, 1),
        "wd3": wd3, "wb3": wb3, "b3t": np.asarray(b3, np.float32).reshape(64, 1),
        "whf": np.ascontiguousarray(np.asarray(Wh1, np.float32)),
        "bh1t": np.asarray(bh1, np.float32).reshape(64, 1),
        "w2t": np.asarray(Wh2, np.float32).reshape(64, 1),
        "bh2t": np.asarray(bh2, np.float32).reshape(1, 1),
        "ident": np.eye(128, dtype=np.float32),
    }
    in_maps = []
    for r in range(n_cores):
        im = dict(common)
        im["stat1"] = np.concatenate(
            [2.0 * xT[:, r * NQ:(r + 1) * NQ], np.ones((1, NQ), np.float32)], axis=0
        ).astype(np.float32)
        im["ew"] = wrap_head(v0[r * EC:(r + 1) * EC], v1[r * EC:(r + 1) * EC])
        in_maps.append(im)
    return in_maps


_CACHE = {}


def _get_program(N, E):
    key = (N, E)
    if key not in _CACHE:
        _CACHE[key] = build_program(N=N, E=E)
    return _CACHE[key]


def kernel(x, edge_index, W1, b1, W2, b2, W3, b3, Wh1, bh1, Wh2, bh2):
    x = np.asarray(x, np.float32)
    ei = np.asarray(edge_index)
    N, E = x.shape[0], ei.shape[1]
    nc = _get_program(N, E)
    in_maps = prepare_inputs(x, ei, W1, b1, W2, b2, W3, b3, Wh1, bh1, Wh2, bh2)
    res = run_bass_kernel_spmd(nc, in_maps, list(range(N_CORES)))
    outs = [np.asarray(res.results[i]["out"], np.float32) for i in range(N_CORES)]
    return np.concatenate(outs)
